# revision 40
# baseline (speedup 1.0000x reference)
"""Trainium2 Bass kernel for nn_MixtureOfExperts (dense MoE, E=8 experts).

Data-parallel over 8 NeuronCores: each core processes B/8 = 2048 tokens with
all expert + gate weights replicated. No collectives.

Per-core algorithm (feature-major intermediates, bf16 matmul operands
with fp32 PSUM accumulation — rel err ~3e-3 vs the 2e-2 gate, and half
the DMA traffic and SBUF footprint of fp32):
  xT = concat(text, tech).T                     # [IN_D, T] host-side marshalling
  gates = softmax(x @ Wg + bg)                  # token-major [T, E]
  oacc  = gates @ b2  (bias matmul, K=E)        # init accumulator
  per expert e:
    hT_e = relu(W1[e].T @ xT + b1[e])           # feature-major [OUT_D, T]
    y_e  = hT_e.T @ W2[e]                       # token-major tiles, PSUM
    oacc += gates[:, e] * y_e                   # fused DVE scalar_tensor_tensor

Schedule notes: expert weights are loaded as one tile per expert
(sub-sliced DMAs so compute starts on the first slice), xt lands as one
coalesced DMA per 512-token half, and the whole gating phase (PE-side:
logit matmuls, bf16 transposes, bias matmuls) runs while the first
expert's weights stream in. Small consts (wg/ones/bg/b2) arrive as ONE
host-packed bf16 tensor — each extra DMA costs ~625ns of serialized
HWDGE descriptor generation. Bias-init copies are issued on DVE before
any expert STT so they drain in DVE's idle window; out-writes ride the
Pool software DGE (SP queue for the final chunk's drain).
"""
import numpy as np
from contextlib import ExitStack

import concourse.bass as bass
import concourse.mybir as mybir
import concourse.tile as tile
from concourse import bacc
from concourse.bass_utils import run_bass_kernel_spmd

B, TEXT_D, TECH_D = 16384, 768, 256
IN_D, OUT_D, E = 1024, 1024, 8
NCORES = 8
T = B // NCORES          # 2048 tokens per core
C = 1024                 # token chunk per outer pass
NCH = T // C             # 2 chunks
P = 128
KT = IN_D // P           # 8 k-tiles (also OUT_D // P)
FT = OUT_D // P          # 8 feature tiles
TT = C // P              # 8 token tiles per chunk
NC2 = C // 512           # 2 half-chunks of 512 tokens
F32 = mybir.dt.float32
F32R = mybir.dt.float32r
BF16 = mybir.dt.bfloat16
AF = mybir.ActivationFunctionType
ALU = mybir.AluOpType
AX = mybir.AxisListType


def build_kernel(nc: bass.Bass, reps: int = 1, loop_n=None, timing: bool = False):
    from concourse.masks import make_identity

    kind_in = "Internal" if timing else "ExternalInput"
    kind_out = "Internal" if timing else "ExternalOutput"
    xt_in = nc.dram_tensor("xt_in", [IN_D, T], BF16, kind=kind_in)
    W1 = nc.dram_tensor("W1", [E, IN_D, OUT_D], BF16, kind=kind_in)
    # b1 arrives host-rearranged: b1r[p, e*FT+f] = b1[e, f*P+p]
    b1 = nc.dram_tensor("b1", [P, E * FT], F32, kind=kind_in)
    W2 = nc.dram_tensor("W2", [E, OUT_D, OUT_D], BF16, kind=kind_in)
    # gcst packs wg (cols 0:64, [p][ko][e]), ones (row 0, 64:192),
    # bg (row 0, 192:200), b2 bf16 (rows 0:8, cols 208:1232)
    gcst_in = nc.dram_tensor("gcst", [P, 1232], BF16, kind=kind_in)
    out = nc.dram_tensor("out", [T, OUT_D], F32, kind=kind_out)
    if timing:
        tin = nc.dram_tensor("tin", [1, 1], F32, kind="ExternalInput")
        tout = nc.dram_tensor("tout", [1, 1], F32, kind="ExternalOutput")

    with tile.TileContext(nc) as tc, ExitStack() as ctx:
        consts = ctx.enter_context(tc.tile_pool(name="consts", bufs=1))
        xt_p = ctx.enter_context(tc.tile_pool(name="xt", bufs=1))
        h_p = ctx.enter_context(tc.tile_pool(name="h", bufs=1))
        oacc_p = ctx.enter_context(tc.tile_pool(name="oacc", bufs=2))
        w1_p = ctx.enter_context(tc.tile_pool(name="w1", bufs=1))
        w2_p = ctx.enter_context(tc.tile_pool(name="w2", bufs=1))
        gates_p = ctx.enter_context(tc.tile_pool(name="gates", bufs=2))
        gt_p = ctx.enter_context(tc.tile_pool(name="gt", bufs=2))
        sm_p = ctx.enter_context(tc.tile_pool(name="sm", bufs=2))
        # PSUM pools: lg 2 + xp 1 + h 2 + y 3 = 8 banks
        pp_lg = ctx.enter_context(tc.tile_pool(name="pp_lg", bufs=2, space="PSUM"))
        pp_xp = ctx.enter_context(tc.tile_pool(name="pp_xp", bufs=1, space="PSUM"))
        pp_h = ctx.enter_context(tc.tile_pool(name="pp_h", bufs=2, space="PSUM"))
        pp_y = ctx.enter_context(tc.tile_pool(name="pp_y", bufs=3, space="PSUM"))

        identity = consts.tile([P, P], BF16)
        make_identity(nc, identity[:])
        # single packed small-consts DMA (each DMA costs ~625ns of serial
        # HWDGE descriptor-gen, so 4 loads -> 1)
        gcst = consts.tile([P, 1232], BF16)
        nc.sync.dma_start(gcst[:], gcst_in[:, :])
        wg_sb = gcst[:, 0:KT * E]
        ones_r = gcst[0:1, 64:64 + P]
        bg_sb = gcst[0:1, 192:200]

        def b2sb(lo, hi):
            return gcst[0:E, 208 + lo:208 + hi]

        # b1 is host-pre-rearranged to [P, E*FT] (a strided gather DMA of
        # the original [E, OUT_D] layout costs ~3.6us on the serial queue)
        b1sb = consts.tile([P, E, FT], F32)
        nc.sync.dma_start(b1sb[:], b1[:, :])

        def load_w1(e):
            w1sb = w1_p.tile([P, KT, OUT_D], BF16, tag="w1")
            for s in range(4):
                fs = bass.ds(s * 256, 256)
                nc.sync.dma_start(
                    w1sb[:, :, fs],
                    W1[e, :, s * 256:(s + 1) * 256].rearrange(
                        "(ko p) f -> p ko f", p=P))
            return w1sb

        def load_w2(e):
            w2sb = w2_p.tile([P, KT, OUT_D], BF16, tag="w2")
            for s in range(4):
                fs = bass.ds(s * 256, 256)
                nc.sync.dma_start(
                    w2sb[:, :, fs],
                    W2[e, :, s * 256:(s + 1) * 256].rearrange(
                        "(ko p) f -> p ko f", p=P))
            return w2sb

        def load_xt(ch, between=None):
            t0 = (ch % NCH) * C
            xt = xt_p.tile([P, KT, C], BF16, tag="xt")
            for lo, hi in ((0, 512), (512, 1024)):
                nc.sync.dma_start(
                    xt[:, :, lo:hi],
                    xt_in[:, t0 + lo:t0 + hi].rearrange("(k p) t -> p k t", p=P))
                if between is not None and hi == 512:
                    between()
            return xt

        def emit_chunk(ch, xt, w1sb_pre, next_xt, last):
            """next_xt: callback issuing the next chunk's xt DMAs right
            before the last expert's layer-2 (they overlap it on the SP
            queue while out-writes ride the Act queue). w1sb_pre: expert
            0's w1 tile when it was preloaded between the xt halves."""
            t0 = (ch % NCH) * C
            gates = gates_p.tile([P, TT, E], BF16, tag="gates")
            oacc = oacc_p.tile([P, TT, OUT_D], F32, tag="oacc")
            out_dma = nc.sync.dma_start if last else nc.gpsimd.dma_start

            w2sb_pre = load_w2(0) if w1sb_pre is not None else None

            # gating for t-tiles [lo, hi): lg matmuls + softmax, with the
            # PE-side transpose+bias of tile t-1 pipelined one tile behind
            # softmax. copy_eng drains the PSUM bias into oacc.
            def gate_bias(t):
                g_t = gates[:, t]
                gtp = pp_xp.tile([P, P], BF16, tag="xp")
                nc.tensor.transpose(gtp[:E, :], g_t, identity[:])
                gt = gt_p.tile([E, P], BF16, tag="gt")
                nc.vector.tensor_copy(gt[:], gtp[:E, :])
                for c2 in range(2):
                    yb = pp_y.tile([P, 512], F32, tag="y")
                    nc.tensor.matmul(yb[:], gt[:],
                                     b2sb(c2 * 512, (c2 + 1) * 512),
                                     start=True, stop=True)
                    dst = oacc[:, t, c2 * 512:(c2 + 1) * 512]
                    nc.vector.tensor_copy(dst, yb[:])

            def gating(lo, hi):
                """lg matmuls + softmax for t-tiles [lo, hi). PE-side
                transpose+bias runs separately via gate_bias, scheduled
                into DMA-wait windows by the caller."""
                for t in range(lo, hi):
                    lg = pp_lg.tile([P, E], F32, tag="lg")
                    nc.tensor.matmul(lg[:], ones_r, bg_sb,
                                     start=True, stop=False)
                    for k in range(KT):
                        nc.tensor.matmul(lg[:], xt[:, k, t * P:(t + 1) * P],
                                         wg_sb[:, k * E:(k + 1) * E],
                                         start=False, stop=(k == KT - 1))
                    nmx = sm_p.tile([P, 1], F32, tag="nmx")
                    nc.vector.reduce_max(nmx[:], lg[:], axis=AX.X)
                    nc.vector.tensor_scalar_mul(nmx[:], nmx[:], -1.0)
                    g_t = gates[:, t]
                    nc.scalar.activation(g_t, lg[:], AF.Exp, bias=nmx[:])
                    sm = sm_p.tile([P, 1], F32, tag="sm")
                    nc.vector.reduce_sum(sm[:], g_t, axis=AX.X)
                    nc.vector.reciprocal(sm[:], sm[:])
                    nc.vector.tensor_scalar_mul(g_t, g_t, sm[:])

            def layer1_half(e, w1sb_e, h, c2, hooks=None):
                cs = bass.ds(c2 * 512, 512)
                for f in range(FT):
                    ph = pp_h.tile([P, 512], F32, tag="ph")
                    for k in range(KT):
                        nc.tensor.matmul(ph[:], w1sb_e[:, k, f * P:(f + 1) * P],
                                         xt[:, k, cs],
                                         start=(k == 0), stop=(k == KT - 1))
                    nc.scalar.activation(h[:, f, cs], ph[:], AF.Relu,
                                         bias=b1sb[:, e, f:f + 1])
                    if hooks and f in hooks:
                        hooks[f]()

            def layer2(e, w2sb_e, h):
                for t in range(TT):
                    y0 = pp_y.tile([P, 512], F32, tag="y")
                    y1 = pp_y.tile([P, 512], F32, tag="y")
                    for k in range(KT):
                        hk = h[:, k, t * P:(t + 1) * P]
                        nc.tensor.matmul(y0[:], hk, w2sb_e[:, k, 0:512],
                                         start=(k == 0), stop=(k == KT - 1))
                        nc.tensor.matmul(y1[:], hk, w2sb_e[:, k, 512:1024],
                                         start=(k == 0), stop=(k == KT - 1))
                    g_e = gates[:, t, e:e + 1]
                    nc.vector.scalar_tensor_tensor(
                        oacc[:, t, 0:512], y0[:], g_e, oacc[:, t, 0:512],
                        op0=ALU.mult, op1=ALU.add)
                    if e == E - 1:
                        # out-writes ride the Pool engine's software DGE
                        # (its own queue; nothing else contends), split per
                        # half so half 0's DMA overlaps half 1's STT. The
                        # final chunk uses the empty SP queue (faster drain).
                        out_dma(out[t0 + t * P:t0 + (t + 1) * P, 0:512],
                                oacc[:, t, 0:512])
                    nc.vector.scalar_tensor_tensor(
                        oacc[:, t, 512:1024], y1[:], g_e, oacc[:, t, 512:1024],
                        op0=ALU.mult, op1=ALU.add)
                    if e == E - 1:
                        out_dma(out[t0 + t * P:t0 + (t + 1) * P, 512:1024],
                                oacc[:, t, 512:1024])

            # ---- expert 0: gating phases interleaved with layer-1 halves.
            # PE order fills DMA-wait windows: lg(t0-3) while xt_h0 trickles,
            # gate_bias(t0-2) while w1 slice 0 lands, layer-1 half 0, then
            # the xt_h1-dependent work, layer-1 half 1, remaining bias.
            w1sb_e = w1sb_pre if w1sb_pre is not None else load_w1(0)
            w2sb_e = w2sb_pre if w2sb_pre is not None else load_w2(0)
            h = h_p.tile([P, FT, C], BF16, tag="h")
            gating(0, 4)
            for t in range(3):
                gate_bias(t)
            layer1_half(0, w1sb_e, h, 0, hooks={0: lambda: gate_bias(3)})
            gating(4, TT)
            layer1_half(0, w1sb_e, h, 1,
                        hooks={f: (lambda t=f + 4: gate_bias(t))
                               for f in range(4)})
            layer2(0, w2sb_e, h)

            # ---- experts 1..7 ----
            for e in range(1, E):
                w1sb_e = load_w1(e)
                w2sb_e = load_w2(e)
                h = h_p.tile([P, FT, C], BF16, tag="h")
                layer1_half(e, w1sb_e, h, 0)
                layer1_half(e, w1sb_e, h, 1)
                if e == E - 1:
                    next_xt()
                layer2(e, w2sb_e, h)

        def emit_body():
            nchunks = NCH * reps
            pre = {}
            xt = load_xt(0, between=lambda: pre.__setitem__('w1', load_w1(0)))
            for ch in range(nchunks):
                nxt = {}

                def next_xt(ch=ch):
                    if ch + 1 < nchunks:
                        nxt['xt'] = load_xt(ch + 1)

                emit_chunk(ch, xt, pre.pop('w1', None), next_xt,
                           ch + 1 == nchunks)
                xt = nxt.get('xt')

        if loop_n is None:
            emit_body()
        else:
            tin_sb = consts.tile([1, 1], F32)
            nc.sync.dma_start(tin_sb[:], tin[:, :])
            with tc.For_i(0, loop_n):
                emit_body()
            nc.sync.dma_start(tout[:, :], tin_sb[:])
    return nc




_compiled = {}


def _get_compiled(reps: int = 1, loop_n=None, timing: bool = False):
    key = (reps, loop_n, timing)
    if key not in _compiled:
        nc = bacc.Bacc(None, target_bir_lowering=False)
        build_kernel(nc, reps, loop_n=loop_n, timing=timing)
        nc.finalize()
        _compiled[key] = nc
    return _compiled[key]


LAST_RESULTS = None


def make_in_maps(np_inputs):
    import ml_dtypes
    bf16 = ml_dtypes.bfloat16
    text_features = np.asarray(np_inputs["text_features"], dtype=np.float32)
    technical_features = np.asarray(
        np_inputs["technical_features"], dtype=np.float32)
    xt_full = np.ascontiguousarray(
        np.concatenate([text_features, technical_features], axis=1).T
        .astype(bf16))
    W1 = np.ascontiguousarray(
        np.asarray(np_inputs["W1"], dtype=np.float32).astype(bf16))
    # b1r[p, e*FT+f] = b1[e, f*P+p] — matches the [P, E, FT] SBUF tile
    b1 = np.ascontiguousarray(
        np.asarray(np_inputs["b1"], dtype=np.float32)
        .reshape(E, FT, P).transpose(2, 0, 1).reshape(P, E * FT))
    W2 = np.ascontiguousarray(
        np.asarray(np_inputs["W2"], dtype=np.float32).astype(bf16))
    b2 = np.asarray(np_inputs["b2"], dtype=np.float32)
    Wg = np.asarray(np_inputs["Wg"], dtype=np.float32)
    bg = np.asarray(np_inputs["bg"], dtype=np.float32).reshape(E)
    gcst = np.zeros((P, 1232), dtype=bf16)
    gcst[:, 0:KT * E] = Wg.reshape(KT, P, E).transpose(1, 0, 2).reshape(P, -1)
    gcst[0, 64:64 + P] = 1.0
    gcst[0, 192:200] = bg
    gcst[0:E, 208:208 + OUT_D] = b2
    gcst = np.ascontiguousarray(gcst)

    in_maps = []
    for i in range(NCORES):
        sl = slice(i * T, (i + 1) * T)
        in_maps.append({
            "xt_in": np.ascontiguousarray(xt_full[:, sl]),
            "W1": W1, "b1": b1, "W2": W2, "gcst": gcst,
        })
    return in_maps


def kernel(text_features, technical_features, W1, b1, W2, b2, Wg, bg):
    global LAST_RESULTS
    nc = _get_compiled()
    in_maps = make_in_maps(dict(
        text_features=text_features, technical_features=technical_features,
        W1=W1, b1=b1, W2=W2, b2=b2, Wg=Wg, bg=bg))
    last_exc = None
    for attempt in range(3):
        try:
            LAST_RESULTS = run_bass_kernel_spmd(nc, in_maps, core_ids=list(range(NCORES)))
            break
        except Exception as e:  # transient device/transfer errors: retry
            last_exc = e
            import time
            time.sleep(2.0 * (attempt + 1))
    else:
        raise last_exc
    return np.concatenate(
        [LAST_RESULTS.results[i]["out"] for i in range(NCORES)], axis=0)


# revision 43
# speedup vs baseline: 1.3919x; 1.3919x over previous
"""Trainium2 Bass kernel for nn_MixtureOfExperts (dense MoE, E=8 experts).

Data-parallel over 8 NeuronCores: each core processes B/8 = 2048 tokens with
all expert + gate weights replicated. No collectives.

Per-core algorithm (feature-major intermediates, bf16 matmul operands
with fp32 PSUM accumulation — rel err ~3e-3 vs the 2e-2 gate, and half
the DMA traffic and SBUF footprint of fp32):
  xT = concat(text, tech).T                     # [IN_D, T] host-side marshalling
  gates = softmax(x @ Wg + bg)                  # token-major [T, E]
  oacc  = gates @ b2  (bias matmul, K=E)        # init accumulator
  per expert e:
    hT_e = relu(W1[e].T @ xT + b1[e])           # feature-major [OUT_D, T]
    y_e  = hT_e.T @ W2[e]                       # token-major tiles, PSUM
    oacc += gates[:, e] * y_e                   # fused DVE scalar_tensor_tensor

Schedule notes: expert weights are loaded as one tile per expert
(sub-sliced DMAs so compute starts on the first slice), xt lands as one
coalesced DMA per 512-token half, and the whole gating phase (PE-side:
logit matmuls, bf16 transposes, bias matmuls) runs while the first
expert's weights stream in. Small consts (wg/ones/bg/b2) arrive as ONE
host-packed bf16 tensor — each extra DMA costs ~625ns of serialized
HWDGE descriptor generation. Bias-init copies are issued on DVE before
any expert STT so they drain in DVE's idle window; out-writes ride the
Pool software DGE (SP queue for the final chunk's drain).
"""
import numpy as np
from contextlib import ExitStack

import concourse.bass as bass
import concourse.mybir as mybir
import concourse.tile as tile
from concourse import bacc
from concourse.bass_utils import run_bass_kernel_spmd

B, TEXT_D, TECH_D = 16384, 768, 256
IN_D, OUT_D, E = 1024, 1024, 8
NCORES = 8
T = B // NCORES          # 2048 tokens per core
C = 1024                 # token chunk per outer pass
NCH = T // C             # 2 chunks
P = 128
KT = IN_D // P           # 8 k-tiles (also OUT_D // P)
FT = OUT_D // P          # 8 feature tiles
TT = C // P              # 8 token tiles per chunk
NC2 = C // 512           # 2 half-chunks of 512 tokens
F32 = mybir.dt.float32
F32R = mybir.dt.float32r
BF16 = mybir.dt.bfloat16
AF = mybir.ActivationFunctionType
ALU = mybir.AluOpType
AX = mybir.AxisListType


def build_kernel(nc: bass.Bass, reps: int = 1, loop_n=None, timing: bool = False):
    from concourse.masks import make_identity

    kind_in = "Internal" if timing else "ExternalInput"
    kind_out = "Internal" if timing else "ExternalOutput"
    xt_in = nc.dram_tensor("xt_in", [IN_D, T], BF16, kind=kind_in)
    W1 = nc.dram_tensor("W1", [E, IN_D, OUT_D], BF16, kind=kind_in)
    # b1 arrives host-rearranged: b1r[p, e*FT+f] = b1[e, f*P+p]
    b1 = nc.dram_tensor("b1", [P, E * FT], F32, kind=kind_in)
    W2 = nc.dram_tensor("W2", [E, OUT_D, OUT_D], BF16, kind=kind_in)
    # gcst packs wg (cols 0:64, [p][ko][e]), ones (row 0, 64:192),
    # bg (row 0, 192:200), b2 bf16 (rows 0:8, cols 208:1232)
    gcst_in = nc.dram_tensor("gcst", [P, 1232], BF16, kind=kind_in)
    out = nc.dram_tensor("out", [T, OUT_D], F32, kind=kind_out)
    if timing:
        tin = nc.dram_tensor("tin", [1, 1], F32, kind="ExternalInput")
        tout = nc.dram_tensor("tout", [1, 1], F32, kind="ExternalOutput")

    with tile.TileContext(nc) as tc, ExitStack() as ctx:
        consts = ctx.enter_context(tc.tile_pool(name="consts", bufs=1))
        xt_p = ctx.enter_context(tc.tile_pool(name="xt", bufs=1))
        h_p = ctx.enter_context(tc.tile_pool(name="h", bufs=1))
        oacc_p = ctx.enter_context(tc.tile_pool(name="oacc", bufs=2))
        w1_p = ctx.enter_context(tc.tile_pool(name="w1", bufs=1))
        w2_p = ctx.enter_context(tc.tile_pool(name="w2", bufs=1))
        gates_p = ctx.enter_context(tc.tile_pool(name="gates", bufs=2))
        gt_p = ctx.enter_context(tc.tile_pool(name="gt", bufs=2))
        sm_p = ctx.enter_context(tc.tile_pool(name="sm", bufs=2))
        # PSUM pools: lg 2 + xp 1 + h 2 + y 3 = 8 banks
        pp_lg = ctx.enter_context(tc.tile_pool(name="pp_lg", bufs=2, space="PSUM"))
        pp_xp = ctx.enter_context(tc.tile_pool(name="pp_xp", bufs=1, space="PSUM"))
        pp_h = ctx.enter_context(tc.tile_pool(name="pp_h", bufs=2, space="PSUM"))
        pp_y = ctx.enter_context(tc.tile_pool(name="pp_y", bufs=3, space="PSUM"))

        identity = consts.tile([P, P], BF16)
        make_identity(nc, identity[:])
        # single packed small-consts DMA (each DMA costs ~625ns of serial
        # HWDGE descriptor-gen, so 4 loads -> 1)
        gcst = consts.tile([P, 1232], BF16)
        nc.sync.dma_start(gcst[:], gcst_in[:, :])
        wg_sb = gcst[:, 0:KT * E]
        ones_r = gcst[0:1, 64:64 + P]
        bg_sb = gcst[0:1, 192:200]

        def b2sb(lo, hi):
            return gcst[0:E, 208 + lo:208 + hi]

        # b1 is host-pre-rearranged to [P, E*FT] (a strided gather DMA of
        # the original [E, OUT_D] layout costs ~3.6us on the serial queue)
        b1sb = consts.tile([P, E, FT], F32)
        nc.sync.dma_start(b1sb[:], b1[:, :])

        def load_w1(e):
            w1sb = w1_p.tile([P, KT, OUT_D], BF16, tag="w1")
            for s in range(4):
                fs = bass.ds(s * 256, 256)
                nc.sync.dma_start(
                    w1sb[:, :, fs],
                    W1[e, :, s * 256:(s + 1) * 256].rearrange(
                        "(ko p) f -> p ko f", p=P))
            return w1sb

        def load_w2(e):
            w2sb = w2_p.tile([P, KT, OUT_D], BF16, tag="w2")
            for s in range(4):
                fs = bass.ds(s * 256, 256)
                nc.sync.dma_start(
                    w2sb[:, :, fs],
                    W2[e, :, s * 256:(s + 1) * 256].rearrange(
                        "(ko p) f -> p ko f", p=P))
            return w2sb

        def load_xt(ch, between=None):
            t0 = (ch % NCH) * C
            xt = xt_p.tile([P, KT, C], BF16, tag="xt")
            for lo, hi in ((0, 512), (512, 1024)):
                nc.sync.dma_start(
                    xt[:, :, lo:hi],
                    xt_in[:, t0 + lo:t0 + hi].rearrange("(k p) t -> p k t", p=P))
                if between is not None and hi == 512:
                    between()
            return xt

        def emit_chunk(ch, xt, w1sb_pre, next_xt, last):
            """next_xt: callback issuing the next chunk's xt DMAs right
            before the last expert's layer-2 (they overlap it on the SP
            queue while out-writes ride the Act queue). w1sb_pre: expert
            0's w1 tile when it was preloaded between the xt halves."""
            t0 = (ch % NCH) * C
            gates = gates_p.tile([P, TT, E], BF16, tag="gates")
            oacc = oacc_p.tile([P, TT, OUT_D], F32, tag="oacc")
            out_dma = nc.sync.dma_start if last else nc.gpsimd.dma_start

            w2sb_pre = load_w2(0) if w1sb_pre is not None else None

            # gating for t-tiles [lo, hi): lg matmuls + softmax, with the
            # PE-side transpose+bias of tile t-1 pipelined one tile behind
            # softmax. copy_eng drains the PSUM bias into oacc.
            def gate_bias(t):
                g_t = gates[:, t]
                gtp = pp_xp.tile([P, P], BF16, tag="xp")
                nc.tensor.transpose(gtp[:E, :], g_t, identity[:])
                gt = gt_p.tile([E, P], BF16, tag="gt")
                nc.vector.tensor_copy(gt[:], gtp[:E, :])
                for c2 in range(2):
                    yb = pp_y.tile([P, 512], F32, tag="y")
                    nc.tensor.matmul(yb[:], gt[:],
                                     b2sb(c2 * 512, (c2 + 1) * 512),
                                     start=True, stop=True)
                    dst = oacc[:, t, c2 * 512:(c2 + 1) * 512]
                    nc.vector.tensor_copy(dst, yb[:])

            def gating(lo, hi):
                """lg matmuls + softmax for t-tiles [lo, hi). PE-side
                transpose+bias runs separately via gate_bias, scheduled
                into DMA-wait windows by the caller."""
                for t in range(lo, hi):
                    lg = pp_lg.tile([P, E], F32, tag="lg")
                    nc.tensor.matmul(lg[:], ones_r, bg_sb,
                                     start=True, stop=False)
                    for k in range(KT):
                        nc.tensor.matmul(lg[:], xt[:, k, t * P:(t + 1) * P],
                                         wg_sb[:, k * E:(k + 1) * E],
                                         start=False, stop=(k == KT - 1))
                    nmx = sm_p.tile([P, 1], F32, tag="nmx")
                    nc.vector.reduce_max(nmx[:], lg[:], axis=AX.X)
                    nc.vector.tensor_scalar_mul(nmx[:], nmx[:], -1.0)
                    g_t = gates[:, t]
                    nc.scalar.activation(g_t, lg[:], AF.Exp, bias=nmx[:])
                    sm = sm_p.tile([P, 1], F32, tag="sm")
                    nc.vector.reduce_sum(sm[:], g_t, axis=AX.X)
                    nc.vector.reciprocal(sm[:], sm[:])
                    nc.vector.tensor_scalar_mul(g_t, g_t, sm[:])

            def layer1_half(e, w1sb_e, h, c2, hooks=None):
                cs = bass.ds(c2 * 512, 512)
                for f in range(FT):
                    ph = pp_h.tile([P, 512], F32, tag="ph")
                    for k in range(KT):
                        nc.tensor.matmul(ph[:], w1sb_e[:, k, f * P:(f + 1) * P],
                                         xt[:, k, cs],
                                         start=(k == 0), stop=(k == KT - 1))
                    nc.scalar.activation(h[:, f, cs], ph[:], AF.Relu,
                                         bias=b1sb[:, e, f:f + 1])
                    if hooks and f in hooks:
                        hooks[f]()

            def layer2(e, w2sb_e, h):
                for t in range(TT):
                    y0 = pp_y.tile([P, 512], F32, tag="y")
                    y1 = pp_y.tile([P, 512], F32, tag="y")
                    for k in range(KT):
                        hk = h[:, k, t * P:(t + 1) * P]
                        nc.tensor.matmul(y0[:], hk, w2sb_e[:, k, 0:512],
                                         start=(k == 0), stop=(k == KT - 1))
                        nc.tensor.matmul(y1[:], hk, w2sb_e[:, k, 512:1024],
                                         start=(k == 0), stop=(k == KT - 1))
                    g_e = gates[:, t, e:e + 1]
                    nc.vector.scalar_tensor_tensor(
                        oacc[:, t, 0:512], y0[:], g_e, oacc[:, t, 0:512],
                        op0=ALU.mult, op1=ALU.add)
                    if e == E - 1:
                        # out-writes ride the Pool engine's software DGE
                        # (its own queue; nothing else contends), split per
                        # half so half 0's DMA overlaps half 1's STT. The
                        # final chunk uses the empty SP queue (faster drain).
                        out_dma(out[t0 + t * P:t0 + (t + 1) * P, 0:512],
                                oacc[:, t, 0:512])
                    nc.vector.scalar_tensor_tensor(
                        oacc[:, t, 512:1024], y1[:], g_e, oacc[:, t, 512:1024],
                        op0=ALU.mult, op1=ALU.add)
                    if e == E - 1:
                        out_dma(out[t0 + t * P:t0 + (t + 1) * P, 512:1024],
                                oacc[:, t, 512:1024])

            # ---- expert 0: gating phases interleaved with layer-1 halves.
            # PE order fills DMA-wait windows: lg(t0-3) while xt_h0 trickles,
            # gate_bias(t0-2) while w1 slice 0 lands, layer-1 half 0, then
            # the xt_h1-dependent work, layer-1 half 1, remaining bias.
            w1sb_e = w1sb_pre if w1sb_pre is not None else load_w1(0)
            w2sb_e = w2sb_pre if w2sb_pre is not None else load_w2(0)
            h = h_p.tile([P, FT, C], BF16, tag="h")
            gating(0, 4)
            for t in range(3):
                gate_bias(t)
            layer1_half(0, w1sb_e, h, 0, hooks={0: lambda: gate_bias(3)})
            gating(4, TT)
            layer1_half(0, w1sb_e, h, 1,
                        hooks={f: (lambda t=f + 4: gate_bias(t))
                               for f in range(4)})
            layer2(0, w2sb_e, h)

            # ---- experts 1..7 ----
            for e in range(1, E):
                w1sb_e = load_w1(e)
                w2sb_e = load_w2(e)
                h = h_p.tile([P, FT, C], BF16, tag="h")
                layer1_half(e, w1sb_e, h, 0)
                layer1_half(e, w1sb_e, h, 1)
                if e == E - 1:
                    next_xt()
                layer2(e, w2sb_e, h)

        def emit_body():
            nchunks = NCH * reps
            pre = {}
            xt = load_xt(0, between=lambda: pre.__setitem__('w1', load_w1(0)))
            for ch in range(nchunks):
                nxt = {}

                def next_xt(ch=ch):
                    if ch + 1 < nchunks:
                        nxt['xt'] = load_xt(ch + 1)

                emit_chunk(ch, xt, pre.pop('w1', None), next_xt,
                           ch + 1 == nchunks)
                xt = nxt.get('xt')

        if loop_n is None:
            emit_body()
        else:
            tin_sb = consts.tile([1, 1], F32)
            nc.sync.dma_start(tin_sb[:], tin[:, :])
            with tc.For_i(0, loop_n):
                emit_body()
            nc.sync.dma_start(tout[:, :], tin_sb[:])
    return nc




_compiled = {}


def _get_compiled(reps: int = 1, loop_n=None, timing: bool = False):
    key = (reps, loop_n, timing)
    if key not in _compiled:
        nc = bacc.Bacc(None, target_bir_lowering=False)
        build_kernel(nc, reps, loop_n=loop_n, timing=timing)
        nc.finalize()
        _compiled[key] = nc
    return _compiled[key]


LAST_RESULTS = None


def make_in_maps(np_inputs):
    import ml_dtypes
    bf16 = ml_dtypes.bfloat16
    text_features = np.asarray(np_inputs["text_features"], dtype=np.float32)
    technical_features = np.asarray(
        np_inputs["technical_features"], dtype=np.float32)
    xt_full = np.ascontiguousarray(
        np.concatenate([text_features, technical_features], axis=1).T
        .astype(bf16))
    W1 = np.ascontiguousarray(
        np.asarray(np_inputs["W1"], dtype=np.float32).astype(bf16))
    # b1r[p, e*FT+f] = b1[e, f*P+p] — matches the [P, E, FT] SBUF tile
    b1 = np.ascontiguousarray(
        np.asarray(np_inputs["b1"], dtype=np.float32)
        .reshape(E, FT, P).transpose(2, 0, 1).reshape(P, E * FT))
    W2 = np.ascontiguousarray(
        np.asarray(np_inputs["W2"], dtype=np.float32).astype(bf16))
    b2 = np.asarray(np_inputs["b2"], dtype=np.float32)
    Wg = np.asarray(np_inputs["Wg"], dtype=np.float32)
    bg = np.asarray(np_inputs["bg"], dtype=np.float32).reshape(E)
    gcst = np.zeros((P, 1232), dtype=bf16)
    gcst[:, 0:KT * E] = Wg.reshape(KT, P, E).transpose(1, 0, 2).reshape(P, -1)
    gcst[0, 64:64 + P] = 1.0
    gcst[0, 192:200] = bg
    gcst[0:E, 208:208 + OUT_D] = b2
    gcst = np.ascontiguousarray(gcst)

    in_maps = []
    for i in range(NCORES):
        sl = slice(i * T, (i + 1) * T)
        in_maps.append({
            "xt_in": np.ascontiguousarray(xt_full[:, sl]),
            "W1": W1, "b1": b1, "W2": W2, "gcst": gcst,
        })
    return in_maps


def kernel(text_features, technical_features, W1, b1, W2, b2, Wg, bg):
    global LAST_RESULTS
    nc = _get_compiled()
    in_maps = make_in_maps(dict(
        text_features=text_features, technical_features=technical_features,
        W1=W1, b1=b1, W2=W2, b2=b2, Wg=Wg, bg=bg))
    last_exc = None
    for attempt in range(3):
        try:
            LAST_RESULTS = run_bass_kernel_spmd(nc, in_maps, core_ids=list(range(NCORES)))
            break
        except Exception as e:  # transient device/transfer errors: retry
            last_exc = e
            import time
            time.sleep(2.0 * (attempt + 1))
    else:
        raise last_exc
    return np.concatenate(
        [LAST_RESULTS.results[i]["out"] for i in range(NCORES)], axis=0)
